# revision 2
# baseline (speedup 1.0000x reference)
"""Multi-head causal attention (b=4, n=2048, d=1024, h=16) on 8 TRN2 cores.

Sharding: core c = (batch b = c//2, head-group g = c%2); each head-group is 8
heads = 512 of the 1024 model dims. QKV weights column-sharded, Wo row-sharded;
host sums the two head-group partial outputs per batch and adds the bias.

Per-core layout trick: everything is kept in "transposed" orientation so each
matmul feeds the next without any on-chip transposes:
  QT/KT [dout, tok] = W.T @ xT        (lhsT = W as stored, rhs = xT)
  scoresT [kv, q]   = KT_h.T @ QT_h   (contraction over head-dim, K=64,
                                       2 heads row-packed in the PE array)
  attnT             = exp(scoresT/8)  (ACT, PSUM->SBUF bf16; no max-subtraction:
                                       |scores/8| < ~2 for this input dist)
  causal mask       = gpsimd.affine_select zeroing attnT above the diagonal
  ctxT [hd, q]      = V_h'.T @ attnT  (V_h' has a ones column appended, so PSUM
                                       row 64 accumulates the softmax denom)
  normalize         = DVE reciprocal_approx + gpsimd partition-broadcast +
                      DVE multiply at PSUM->SBUF copyback
  out [tok, dout]   = ctxT.T @ Wo     (partial over this head-group's 512 dims)

v2 scheduling changes (baseline measured ~301.5 us HW exec):
- input DMAs: weights land as single 3D-AP DMAs (1 instr each instead of
  8/4); xT chunk 0 fine-grained on the sync queue, chunks 1-3 merged per
  k-tile on the SCALAR queue (scalar is a HWDGE engine and idle at the
  head) so issue serialization no longer gates the first projections.
- 20 dummy warm-up matmuls at t=0 keep the PE HAM activity monitor busy
  through the DMA-bound head so real work starts at 2.4 GHz, not 1.2.
- qc=2 / qc=3 attention blocks are interleaved (and outproj(1) moved into
  the qc=3 window) so the ACT-bound late chunks always have PE filler.
- the last attention block carries 2 out-proj units of the final chunk as
  in-block partial accumulations (dt2=0..2 early, dt2=3 after the norm),
  shrinking the tail; tail copybacks alternate DVE/ACT engines.
"""

import sys

if "/opt/trn_rl_repo" not in sys.path:
    sys.path.insert(0, "/opt/trn_rl_repo")

import numpy as np
import ml_dtypes

import concourse.bacc as bacc
import concourse.mybir as mybir
import concourse.tile as tile
from concourse import bass_utils

N_CORES = 8
B = 4          # batch
N = 2048       # sequence length
D = 1024       # model dim
H = 16         # total heads
HD = 64        # head dim
HH = 8         # heads per core
DH = 512       # model dims per core (HH * HD)
N_DT = 4       # 128-row d-tiles of DH (one head pair each)
N_QC = 4       # 512-wide query chunks
N_KT = 16      # 128-wide kv token tiles
N_TT = 16      # 128-wide token tiles
BF16 = mybir.dt.bfloat16
F32 = mybir.dt.float32
AF = mybir.ActivationFunctionType


def _emit(nc, tc, xt_d, wq_d, wk_d, wv_d, wo_d, out_d):
    import contextlib

    ctx = contextlib.ExitStack()
    with ctx:
        const = ctx.enter_context(tc.tile_pool(name="const", bufs=1))
        ps = ctx.enter_context(tc.tile_pool(name="ps", bufs=2, space="PSUM"))
        attn_pool = ctx.enter_context(tc.tile_pool(name="attn", bufs=12))
        small = ctx.enter_context(tc.tile_pool(name="small", bufs=3))
        outp = ctx.enter_context(tc.tile_pool(name="outp", bufs=4))

        # ---- PE warm-up ----
        # the input load is DMA-bound for the first ~15us; dummy matmuls keep
        # the HAM activity window busy so the PE is at 2.4 GHz (K=8/8) when
        # the first projection lands.  They use the "ps" psum tag, whose
        # first real user (qc=0 scores) is ~20us later.
        dummy = const.tile([128, 512], BF16, name="dummy", tag="dummy")
        nc.vector.memset(dummy[:], 0.0)
        for _ in range(20):
            pw = ps.tile([128, 512], F32, name="warm", tag="ps")
            nc.tensor.matmul(pw[:], dummy[:, 0:128], dummy[:], start=True, stop=True)

        # ---- input DMAs ----
        # weights as single 3D-AP DMAs; xT chunk 0 per k-tile (sync queue) so
        # the first projection can start early; xT chunks 1-3 merged per
        # k-tile on the scalar queue (also a HWDGE engine, idle at the head).
        wq_all = const.tile([128, 8 * DH], BF16, name="wq", tag="wq")
        wk_all = const.tile([128, 8 * DH], BF16, name="wk", tag="wk")
        wv_all = const.tile([128, 8 * DH], BF16, name="wv", tag="wv")
        wo_all = const.tile([128, 4 * D], BF16, name="wo", tag="wo")
        xt = [const.tile([128, N], BF16, name=f"xt{k}", tag=f"xt{k}") for k in range(8)]
        xt_v = xt_d.ap().rearrange("(t p) n -> t p n", p=128)

        nc.sync.dma_start(
            wq_all.rearrange("p (k n) -> p k n", n=DH),
            wq_d.ap().rearrange("(k p) n -> p k n", p=128),
        )
        for k in range(8):
            nc.sync.dma_start(xt[k][:, 0:512], xt_v[k][:, 0:512])
        nc.sync.dma_start(
            wo_all.rearrange("p (k n) -> p k n", n=D),
            wo_d.ap().rearrange("(k p) n -> p k n", p=128),
        )
        nc.scalar.dma_start(
            wk_all.rearrange("p (k n) -> p k n", n=DH),
            wk_d.ap().rearrange("(k p) n -> p k n", p=128),
        )
        nc.scalar.dma_start(
            wv_all.rearrange("p (k n) -> p k n", n=DH),
            wv_d.ap().rearrange("(k p) n -> p k n", p=128),
        )
        for k in range(8):
            nc.scalar.dma_start(xt[k][:, 512:2048], xt_v[k][:, 512:2048])

        def wq_k(k):
            return wq_all[:, k * DH:(k + 1) * DH]

        def wk_k(k):
            return wk_all[:, k * DH:(k + 1) * DH]

        def wv_k(k):
            return wv_all[:, k * DH:(k + 1) * DH]

        def wo_k(k):
            return wo_all[:, k * D:(k + 1) * D]

        # ---- persistent intermediates ----
        qt = [const.tile([128, N], BF16, name=f"qt{k}", tag=f"qt{k}") for k in range(N_DT)]
        kt = [const.tile([128, N], BF16, name=f"kt{k}", tag=f"kt{k}") for k in range(N_DT)]
        # V' per token tile: 4 head-pair groups of [V_even(64) | 1 | V_odd(64) | 1]
        vp = [const.tile([128, 520], BF16, name=f"vp{k}", tag=f"vp{k}") for k in range(N_TT)]
        cxt = [const.tile([128, N], BF16, name=f"cxt{k}", tag=f"cxt{k}") for k in range(N_DT)]

        # ones columns of V' (offsets 64 + 65*k cover both ones cols of each pair)
        for t in range(N_TT):
            nc.vector.memset(vp[t][:, 64:520:65], 1.0)

        # ---- projections for one token chunk, one dt/tt piece (1/4) ----
        def emit_proj_piece(tc_i, dt):
            csl = slice(tc_i * 512, (tc_i + 1) * 512)
            dsl = slice(dt * 128, (dt + 1) * 128)
            pq = ps.tile([128, 512], F32, name="pq", tag="po", bufs=4)
            for k in range(8):
                nc.tensor.matmul(
                    pq[:], wq_k(k)[:, dsl], xt[k][:, csl], start=(k == 0), stop=(k == 7)
                )
            nc.vector.tensor_copy(qt[dt][:, csl], pq[:])
            pk = ps.tile([128, 512], F32, name="pk", tag="po", bufs=4)
            for k in range(8):
                nc.tensor.matmul(
                    pk[:], wk_k(k)[:, dsl], xt[k][:, csl], start=(k == 0), stop=(k == 7)
                )
            nc.vector.tensor_copy(kt[dt][:, csl], pk[:])
            tt = tc_i * 4 + dt
            tsl = slice(tt * 128, (tt + 1) * 128)
            pv = ps.tile([128, 512], F32, name="pv", tag="po", bufs=4)
            for k in range(8):
                nc.tensor.matmul(
                    pv[:], xt[k][:, tsl], wv_k(k)[:, 0:DH], start=(k == 0), stop=(k == 7)
                )
            pv_g = pv.rearrange("p (g c) -> p g c", c=128)
            vp_g = vp[tt].rearrange("p (g c) -> p g c", c=130)
            nc.vector.tensor_copy(vp_g[:, :, 0:64], pv_g[:, :, 0:64])
            nc.vector.tensor_copy(vp_g[:, :, 65:129], pv_g[:, :, 64:128])

        def emit_proj(tc_i):
            for dt in range(N_DT):
                emit_proj_piece(tc_i, dt)

        # ---- attention for one query chunk, one head-pair dt ----
        # fillers: {kv_iter_index: callable} emitted right after that
        # iteration's ctx matmuls (used to pull out-proj work into the
        # ACT-paced final blocks).
        def emit_attn_dt(qc, dt, fillers=None):
            qsl = slice(qc * 512, (qc + 1) * 512)
            if True:
                ea = slice(0, 64)     # even head of the pair: partitions 0:64
                eb = slice(64, 128)   # odd head: partitions 64:128
                va = slice(dt * 130, dt * 130 + 65)        # [V_even | 1]
                vb = slice(dt * 130 + 65, dt * 130 + 130)  # [V_odd | 1]
                ca = ps.tile([65, 512], F32, name="ca", tag="po", bufs=4)
                cb = ps.tile([65, 512], F32, name="cb", tag="po", bufs=4)
                nkt = 4 * (qc + 1)
                # diagonal kv-tiles first: their longer exp->mask->ctx chain
                # then overlaps the independent (unmasked) off-diagonal tiles.
                # Each psum/attn tile holds BOTH heads [A|B] for one kv-tile so
                # a single exp releases the next A+B score matmuls atomically
                # (back-to-back K=64 row-packed pairs overlap ~2x in the PE).
                for i, ktl in enumerate(reversed(range(nkt))):
                    ksl = slice(ktl * 128, ktl * 128 + 128)
                    j = ktl - 4 * qc
                    # diagonal tiles only attend to q >= 128*j within the
                    # chunk: skip the fully-masked q-range entirely. PSUM
                    # accumulation stays correct: start=True clears the whole
                    # bank's has_written bits, and each element's first writer
                    # overwrites (per-element semantics).
                    qoff = 128 * j if j > 0 else 0
                    nw = 512 - qoff
                    qn = slice(qc * 512 + qoff, (qc + 1) * 512)
                    s = ps.tile([128, 1024], F32, name="s", tag="ps")
                    nc.tensor.matmul(s[:, qoff:512], kt[dt][ea, ksl], qt[dt][ea, qn], start=True, stop=True)
                    nc.tensor.matmul(s[:, 512 + qoff:1024], kt[dt][eb, ksl], qt[dt][eb, qn], start=True, stop=True)
                    at = attn_pool.tile([128, 1024], BF16, name="at", tag="attn")
                    s3 = s.rearrange("p (o q) -> p o q", o=2)[:, :, qoff:512]
                    at3 = at.rearrange("p (o q) -> p o q", o=2)[:, :, qoff:512]
                    nc.scalar.activation(at3, s3, AF.Exp, scale=0.125)
                    if j >= 0:
                        # diagonal: zero attn where kv > q (pure triangle after
                        # the qoff shift; both halves = same kv-tile)
                        nc.gpsimd.affine_select(
                            at3,
                            at3,
                            pattern=[[0, 2], [1, nw]],
                            compare_op=mybir.AluOpType.is_ge,
                            fill=0.0,
                            base=0,
                            channel_multiplier=-1,
                        )
                    first = i == 0
                    last = i == nkt - 1
                    nc.tensor.matmul(ca[:, qoff:512], vp[ktl][:, va], at[:, qoff:512], start=first, stop=last)
                    nc.tensor.matmul(cb[:, qoff:512], vp[ktl][:, vb], at[:, 512 + qoff:1024], start=first, stop=last)
                    if fillers and i in fillers:
                        fillers[i]()

                # normalize and copy back to SBUF (bf16)
                # custom-DVE ops don't handle partition-offset inputs; stage the
                # denom row at partition 0 first (builtin copy does remap lanes)
                da = small.tile([1, 512], F32, name="da", tag="d")
                db = small.tile([1, 512], F32, name="db", tag="d")
                nc.vector.tensor_copy(da[:], ca[64:65, :])
                nc.vector.tensor_copy(db[:], cb[64:65, :])
                ra = small.tile([1, 512], F32, name="ra", tag="r")
                rb = small.tile([1, 512], F32, name="rb", tag="r")
                nc.vector.reciprocal_approx_fast(ra[:], da[:])
                nc.vector.reciprocal_approx_fast(rb[:], db[:])
                # broadcast r across 64 partitions on gpsimd (engines are
                # lane-locked; gpsimd has sequencer headroom here)
                rba = small.tile([64, 512], F32, name="rba", tag="rb")
                rbb = small.tile([64, 512], F32, name="rbb", tag="rb")
                nc.gpsimd.partition_broadcast(rba[:], ra[:])
                nc.gpsimd.partition_broadcast(rbb[:], rb[:])
                nc.vector.tensor_mul(cxt[dt][0:64, qsl], ca[0:64, :], rba[:])
                tmpb = small.tile([64, 512], BF16, name="tmpb", tag="tmp")
                nc.vector.tensor_mul(tmpb[:], cb[0:64, :], rbb[:])
                # partition shift 0:64 -> 64:128 (engines are lane-locked; DMA is not)
                nc.sync.dma_start(cxt[dt][64:128, qsl], tmpb[:])

        # ---- out-projection, one (token-tile, n-half) unit ----
        def emit_outproj_unit(qc, u, tag="po", copy_engine="vector"):
            tti, nck = u // 2, u % 2
            tt = qc * 4 + tti
            tsl = slice(tt * 128, (tt + 1) * 128)
            nsl = slice(nck * 512, (nck + 1) * 512)
            po = ps.tile(
                [128, 512], F32, name="po", tag=tag, bufs=(4 if tag == "po" else 2)
            )
            for dt2 in range(N_DT):
                nc.tensor.matmul(
                    po[:], cxt[dt2][:, tsl], wo_k(dt2)[:, nsl],
                    start=(dt2 == 0), stop=(dt2 == 3),
                )
            ob = outp.tile([128, 512], F32, name="ob", tag="ob")
            if copy_engine == "scalar":
                nc.scalar.copy(ob[:], po[:])
            else:
                nc.vector.tensor_copy(ob[:], po[:])
            nc.sync.dma_start(out_d.ap()[tsl, nsl], ob[:])

        # out-proj unit of the final chunk, split: dt2=0..2 accumulate inside
        # the last attention block (psum tile stays open), dt2=3 + copyback
        # after that block's normalization.
        def start_unit3(u):
            tti, nck = u // 2, u % 2
            tt = 12 + tti
            tsl = slice(tt * 128, (tt + 1) * 128)
            nsl = slice(nck * 512, (nck + 1) * 512)
            po = ps.tile([128, 512], F32, name="po3", tag="po", bufs=4)
            for dt2 in range(3):
                nc.tensor.matmul(
                    po[:], cxt[dt2][:, tsl], wo_k(dt2)[:, nsl],
                    start=(dt2 == 0), stop=False,
                )
            return (po, tsl, nsl)

        def finish_unit3(st, copy_engine="vector"):
            po, tsl, nsl = st
            nc.tensor.matmul(po[:], cxt[3][:, tsl], wo_k(3)[:, nsl], start=False, stop=True)
            ob = outp.tile([128, 512], F32, name="ob", tag="ob")
            if copy_engine == "scalar":
                nc.scalar.copy(ob[:], po[:])
            else:
                nc.vector.tensor_copy(ob[:], po[:])
            nc.sync.dma_start(out_d.ap()[tsl, nsl], ob[:])

        # ---- emission schedule ----
        # qc=0/1: proj-rich region, ascending; qc=2/3 attention interleaved so
        # the ACT-heavy late chunks spread over a window with PE filler
        # (proj(3), outproj(1), outproj(2)).
        emit_proj(0)
        for dt in range(N_DT):
            emit_attn_dt(0, dt)
            emit_proj_piece(1, dt)
        for dt in range(N_DT):
            emit_attn_dt(1, dt)
            emit_outproj_unit(0, 2 * dt)
            emit_outproj_unit(0, 2 * dt + 1)
            emit_proj_piece(2, dt)
        emit_attn_dt(2, 0)
        emit_proj_piece(3, 0)
        emit_attn_dt(2, 1)
        emit_proj_piece(3, 1)
        emit_attn_dt(3, 0)
        emit_attn_dt(2, 2)
        emit_proj_piece(3, 2)
        emit_outproj_unit(1, 0)
        emit_outproj_unit(1, 1)
        emit_attn_dt(3, 1)
        emit_outproj_unit(1, 2)
        emit_outproj_unit(1, 3)
        emit_attn_dt(2, 3)
        emit_proj_piece(3, 3)
        emit_outproj_unit(1, 4)
        emit_outproj_unit(1, 5)
        emit_attn_dt(3, 2)
        emit_outproj_unit(1, 6)
        emit_outproj_unit(1, 7)
        for u in range(8):
            emit_outproj_unit(2, u)
        # last attention block: carry 2 final-chunk out-proj partials in the
        # two free "po" slots (ca/cb hold the other two until the norm).
        pend = []
        fill = {
            4: lambda: pend.append(start_unit3(0)),
            8: lambda: pend.append(start_unit3(1)),
        }
        emit_attn_dt(3, 3, fillers=fill)
        finish_unit3(pend[0], copy_engine="vector")
        finish_unit3(pend[1], copy_engine="scalar")
        for u in range(2, 8):
            emit_outproj_unit(
                3,
                u,
                tag=("ps" if u % 2 else "po"),
                copy_engine=("scalar" if u % 2 else "vector"),
            )


def build_bass():
    nc = bacc.Bacc("TRN2", target_bir_lowering=False, debug=False, num_devices=N_CORES)
    xt_d = nc.dram_tensor("xt", (D, N), BF16, kind="ExternalInput")
    wq_d = nc.dram_tensor("wq", (D, DH), BF16, kind="ExternalInput")
    wk_d = nc.dram_tensor("wk", (D, DH), BF16, kind="ExternalInput")
    wv_d = nc.dram_tensor("wv", (D, DH), BF16, kind="ExternalInput")
    wo_d = nc.dram_tensor("wo", (DH, D), BF16, kind="ExternalInput")
    out_d = nc.dram_tensor("out", (N, D), F32, kind="ExternalOutput")
    with tile.TileContext(nc) as tc:
        _emit(nc, tc, xt_d, wq_d, wk_d, wv_d, wo_d, out_d)
    nc.compile()
    return nc


_NC = None


def _get_nc():
    global _NC
    if _NC is None:
        _NC = build_bass()
    return _NC


def make_in_maps(x, Wq, Wk, Wv, Wo):
    bf = ml_dtypes.bfloat16
    in_maps = []
    for c in range(N_CORES):
        b, g = c // 2, c % 2
        gs = slice(g * DH, (g + 1) * DH)
        in_maps.append(
            {
                "xt": np.ascontiguousarray(x[b].T).astype(bf),
                "wq": np.ascontiguousarray(Wq[:, gs]).astype(bf),
                "wk": np.ascontiguousarray(Wk[:, gs]).astype(bf),
                "wv": np.ascontiguousarray(Wv[:, gs]).astype(bf),
                "wo": np.ascontiguousarray(Wo[gs, :]).astype(bf),
            }
        )
    return in_maps


def kernel(x, Wq, Wk, Wv, Wo, bo, _trace=False):
    x = np.asarray(x, dtype=np.float32)
    nc = _get_nc()
    in_maps = make_in_maps(x, Wq, Wk, Wv, Wo)
    res = bass_utils.run_bass_kernel_spmd(
        nc, in_maps, core_ids=list(range(N_CORES)), trace=_trace
    )
    out = np.empty((B, N, D), dtype=np.float32)
    bo32 = np.asarray(bo, dtype=np.float32)
    for b in range(B):
        out[b] = res.results[2 * b]["out"] + res.results[2 * b + 1]["out"] + bo32
    if _trace:
        return out, res
    return out


# revision 5
# speedup vs baseline: 1.0306x; 1.0306x over previous
"""Multi-head causal attention (b=4, n=2048, d=1024, h=16) on 8 TRN2 cores.

Sharding: core c = (batch b = c//2, head-group g = c%2); each head-group is 8
heads = 512 of the 1024 model dims. QKV weights column-sharded, Wo row-sharded;
host sums the two head-group partial outputs per batch and adds the bias.

Per-core layout trick: everything is kept in "transposed" orientation so each
matmul feeds the next without any on-chip transposes:
  QT/KT [dout, tok] = W.T @ xT        (lhsT = W as stored, rhs = xT)
  scoresT [kv, q]   = KT_h.T @ QT_h   (contraction over head-dim, K=64,
                                       2 heads row-packed in the PE array)
  attnT             = exp(scoresT/8)  (ACT, PSUM->SBUF bf16; no max-subtraction:
                                       |scores/8| < ~2 for this input dist)
  causal mask       = gpsimd.affine_select zeroing attnT above the diagonal
  ctxT [hd, q]      = V_h'.T @ attnT  (V_h' has a ones column appended, so PSUM
                                       row 64 accumulates the softmax denom)
  normalize         = DVE reciprocal_approx + gpsimd partition-broadcast +
                      DVE multiply at PSUM->SBUF copyback
  out [tok, dout]   = ctxT.T @ Wo     (partial over this head-group's 512 dims)

v2 scheduling changes (baseline measured ~301.5 us HW exec):
- input DMAs: weights land as single 3D-AP DMAs (1 instr each instead of
  8/4); xT chunk 0 fine-grained on the sync queue, chunks 1-3 merged per
  k-tile on the SCALAR queue (scalar is a HWDGE engine and idle at the
  head) so issue serialization no longer gates the first projections.
- 20 dummy warm-up matmuls at t=0 keep the PE HAM activity monitor busy
  through the DMA-bound head so real work starts at 2.4 GHz, not 1.2.
- qc=2 / qc=3 attention blocks are interleaved (and outproj(1) moved into
  the qc=3 window) so the ACT-bound late chunks always have PE filler.
- the last attention block carries 2 out-proj units of the final chunk as
  in-block partial accumulations (dt2=0..2 early, dt2=3 after the norm),
  shrinking the tail; tail copybacks alternate DVE/ACT engines.
"""

import sys

if "/opt/trn_rl_repo" not in sys.path:
    sys.path.insert(0, "/opt/trn_rl_repo")

import numpy as np
import ml_dtypes

import concourse.bacc as bacc
import concourse.mybir as mybir
import concourse.tile as tile
from concourse import bass_utils

N_CORES = 8
B = 4          # batch
N = 2048       # sequence length
D = 1024       # model dim
H = 16         # total heads
HD = 64        # head dim
HH = 8         # heads per core
DH = 512       # model dims per core (HH * HD)
N_DT = 4       # 128-row d-tiles of DH (one head pair each)
N_QC = 4       # 512-wide query chunks
N_KT = 16      # 128-wide kv token tiles
N_TT = 16      # 128-wide token tiles
BF16 = mybir.dt.bfloat16
F32 = mybir.dt.float32
AF = mybir.ActivationFunctionType


def _emit(nc, tc, xt_d, wq_d, wk_d, wv_d, wo_d, out_d):
    import contextlib

    ctx = contextlib.ExitStack()
    with ctx:
        const = ctx.enter_context(tc.tile_pool(name="const", bufs=1))
        ps = ctx.enter_context(tc.tile_pool(name="ps", bufs=2, space="PSUM"))
        attn_pool = ctx.enter_context(tc.tile_pool(name="attn", bufs=12))
        small = ctx.enter_context(tc.tile_pool(name="small", bufs=3))
        outp = ctx.enter_context(tc.tile_pool(name="outp", bufs=4))

        # ---- PE warm-up ----
        # the input load is DMA-bound for the first ~15us; dummy matmuls keep
        # the HAM activity window busy so the PE is at 2.4 GHz (K=8/8) when
        # the first projection lands.  They use the "ps" psum tag, whose
        # first real user (qc=0 scores) is ~20us later.
        dummy = const.tile([128, 512], BF16, name="dummy", tag="dummy")
        nc.vector.memset(dummy[:], 0.0)
        for _ in range(20):
            pw = ps.tile([128, 512], F32, name="warm", tag="ps")
            nc.tensor.matmul(pw[:], dummy[:, 0:128], dummy[:], start=True, stop=True)

        # ---- input DMAs ----
        # weights first, then xT chunked by token-chunk, so the first
        # projection matmuls (which need all 8 k-tiles of W and of one token
        # chunk of xT) start as early as possible
        def load_w(d, name):
            ts = [
                const.tile([128, DH], BF16, name=f"{name}{k}", tag=f"{name}{k}")
                for k in range(8)
            ]
            v = d.ap().rearrange("(t p) n -> t p n", p=128)
            for k in range(8):
                nc.sync.dma_start(ts[k][:], v[k])
            return ts

        wq = [
            const.tile([128, DH], BF16, name=f"wq{k}", tag=f"wq{k}") for k in range(8)
        ]
        wq_v = wq_d.ap().rearrange("(t p) n -> t p n", p=128)
        xt = [const.tile([128, N], BF16, name=f"xt{k}", tag=f"xt{k}") for k in range(8)]
        xt_v = xt_d.ap().rearrange("(t p) n -> t p n", p=128)
        # interleave so the first projection's k-accumulation can start after
        # the first (wq[k], xt[k]) pair lands instead of after all of them
        for k in range(8):
            nc.sync.dma_start(wq[k][:], wq_v[k])
            nc.sync.dma_start(xt[k][:, 0:512], xt_v[k][:, 0:512])
        wk = load_w(wk_d, "wk")
        wv = load_w(wv_d, "wv")
        for tc_i in range(1, 4):
            for k in range(8):
                csl = slice(tc_i * 512, (tc_i + 1) * 512)
                nc.sync.dma_start(xt[k][:, csl], xt_v[k][:, csl])
        wo = [const.tile([128, D], BF16, name=f"wo{k}", tag=f"wo{k}") for k in range(4)]
        wo_v = wo_d.ap().rearrange("(t p) n -> t p n", p=128)
        for k in range(4):
            nc.sync.dma_start(wo[k][:], wo_v[k])

        def wq_k(k):
            return wq[k]

        def wk_k(k):
            return wk[k]

        def wv_k(k):
            return wv[k]

        def wo_k(k):
            return wo[k]

        # ---- persistent intermediates ----
        qt = [const.tile([128, N], BF16, name=f"qt{k}", tag=f"qt{k}") for k in range(N_DT)]
        kt = [const.tile([128, N], BF16, name=f"kt{k}", tag=f"kt{k}") for k in range(N_DT)]
        # V' per token tile: 4 head-pair groups of [V_even(64) | 1 | V_odd(64) | 1]
        vp = [const.tile([128, 520], BF16, name=f"vp{k}", tag=f"vp{k}") for k in range(N_TT)]
        cxt = [const.tile([128, N], BF16, name=f"cxt{k}", tag=f"cxt{k}") for k in range(N_DT)]

        # ones columns of V' (offsets 64 + 65*k cover both ones cols of each pair)
        for t in range(N_TT):
            nc.vector.memset(vp[t][:, 64:520:65], 1.0)

        # ---- projections for one token chunk, one dt/tt piece (1/4) ----
        def emit_proj_piece(tc_i, dt):
            csl = slice(tc_i * 512, (tc_i + 1) * 512)
            dsl = slice(dt * 128, (dt + 1) * 128)
            pq = ps.tile([128, 512], F32, name="pq", tag="po", bufs=4)
            for k in range(8):
                nc.tensor.matmul(
                    pq[:], wq_k(k)[:, dsl], xt[k][:, csl], start=(k == 0), stop=(k == 7)
                )
            nc.vector.tensor_copy(qt[dt][:, csl], pq[:])
            pk = ps.tile([128, 512], F32, name="pk", tag="po", bufs=4)
            for k in range(8):
                nc.tensor.matmul(
                    pk[:], wk_k(k)[:, dsl], xt[k][:, csl], start=(k == 0), stop=(k == 7)
                )
            nc.vector.tensor_copy(kt[dt][:, csl], pk[:])
            tt = tc_i * 4 + dt
            tsl = slice(tt * 128, (tt + 1) * 128)
            pv = ps.tile([128, 512], F32, name="pv", tag="po", bufs=4)
            for k in range(8):
                nc.tensor.matmul(
                    pv[:], xt[k][:, tsl], wv_k(k)[:, 0:DH], start=(k == 0), stop=(k == 7)
                )
            pv_g = pv.rearrange("p (g c) -> p g c", c=128)
            vp_g = vp[tt].rearrange("p (g c) -> p g c", c=130)
            nc.vector.tensor_copy(vp_g[:, :, 0:64], pv_g[:, :, 0:64])
            nc.vector.tensor_copy(vp_g[:, :, 65:129], pv_g[:, :, 64:128])

        def emit_proj(tc_i):
            for dt in range(N_DT):
                emit_proj_piece(tc_i, dt)

        # ---- attention for one query chunk, one head-pair dt ----
        # fillers: {kv_iter_index: callable} emitted right after that
        # iteration's ctx matmuls (used to pull out-proj work into the
        # ACT-paced final blocks).
        def emit_attn_dt(qc, dt, fillers=None):
            qsl = slice(qc * 512, (qc + 1) * 512)
            if True:
                ea = slice(0, 64)     # even head of the pair: partitions 0:64
                eb = slice(64, 128)   # odd head: partitions 64:128
                va = slice(dt * 130, dt * 130 + 65)        # [V_even | 1]
                vb = slice(dt * 130 + 65, dt * 130 + 130)  # [V_odd | 1]
                ca = ps.tile([65, 512], F32, name="ca", tag="po", bufs=4)
                cb = ps.tile([65, 512], F32, name="cb", tag="po", bufs=4)
                nkt = 4 * (qc + 1)
                # diagonal kv-tiles first: their longer exp->mask->ctx chain
                # then overlaps the independent (unmasked) off-diagonal tiles.
                # Each psum/attn tile holds BOTH heads [A|B] for one kv-tile so
                # a single exp releases the next A+B score matmuls atomically
                # (back-to-back K=64 row-packed pairs overlap ~2x in the PE).
                for i, ktl in enumerate(reversed(range(nkt))):
                    ksl = slice(ktl * 128, ktl * 128 + 128)
                    j = ktl - 4 * qc
                    # diagonal tiles only attend to q >= 128*j within the
                    # chunk: skip the fully-masked q-range entirely. PSUM
                    # accumulation stays correct: start=True clears the whole
                    # bank's has_written bits, and each element's first writer
                    # overwrites (per-element semantics).
                    qoff = 128 * j if j > 0 else 0
                    nw = 512 - qoff
                    qn = slice(qc * 512 + qoff, (qc + 1) * 512)
                    s = ps.tile([128, 1024], F32, name="s", tag="ps")
                    nc.tensor.matmul(s[:, qoff:512], kt[dt][ea, ksl], qt[dt][ea, qn], start=True, stop=True)
                    nc.tensor.matmul(s[:, 512 + qoff:1024], kt[dt][eb, ksl], qt[dt][eb, qn], start=True, stop=True)
                    at = attn_pool.tile([128, 1024], BF16, name="at", tag="attn")
                    s3 = s.rearrange("p (o q) -> p o q", o=2)[:, :, qoff:512]
                    at3 = at.rearrange("p (o q) -> p o q", o=2)[:, :, qoff:512]
                    nc.scalar.activation(at3, s3, AF.Exp, scale=0.125)
                    if j >= 0:
                        # diagonal: zero attn where kv > q (pure triangle after
                        # the qoff shift; both halves = same kv-tile)
                        nc.gpsimd.affine_select(
                            at3,
                            at3,
                            pattern=[[0, 2], [1, nw]],
                            compare_op=mybir.AluOpType.is_ge,
                            fill=0.0,
                            base=0,
                            channel_multiplier=-1,
                        )
                    first = i == 0
                    last = i == nkt - 1
                    nc.tensor.matmul(ca[:, qoff:512], vp[ktl][:, va], at[:, qoff:512], start=first, stop=last)
                    nc.tensor.matmul(cb[:, qoff:512], vp[ktl][:, vb], at[:, 512 + qoff:1024], start=first, stop=last)
                    if fillers and i in fillers:
                        fillers[i]()

                # normalize and copy back to SBUF (bf16)
                # custom-DVE ops don't handle partition-offset inputs; stage the
                # denom row at partition 0 first (builtin copy does remap lanes)
                da = small.tile([1, 512], F32, name="da", tag="d")
                db = small.tile([1, 512], F32, name="db", tag="d")
                nc.vector.tensor_copy(da[:], ca[64:65, :])
                nc.vector.tensor_copy(db[:], cb[64:65, :])
                ra = small.tile([1, 512], F32, name="ra", tag="r")
                rb = small.tile([1, 512], F32, name="rb", tag="r")
                nc.vector.reciprocal_approx_fast(ra[:], da[:])
                nc.vector.reciprocal_approx_fast(rb[:], db[:])
                # broadcast r across 64 partitions on gpsimd (engines are
                # lane-locked; gpsimd has sequencer headroom here)
                rba = small.tile([64, 512], F32, name="rba", tag="rb")
                rbb = small.tile([64, 512], F32, name="rbb", tag="rb")
                nc.gpsimd.partition_broadcast(rba[:], ra[:])
                nc.gpsimd.partition_broadcast(rbb[:], rb[:])
                nc.vector.tensor_mul(cxt[dt][0:64, qsl], ca[0:64, :], rba[:])
                tmpb = small.tile([64, 512], BF16, name="tmpb", tag="tmp")
                nc.vector.tensor_mul(tmpb[:], cb[0:64, :], rbb[:])
                # partition shift 0:64 -> 64:128 (engines are lane-locked; DMA is not)
                nc.sync.dma_start(cxt[dt][64:128, qsl], tmpb[:])

        # ---- out-projection, one (token-tile, n-half) unit ----
        def emit_outproj_unit(qc, u, tag="po", copy_engine="vector"):
            tti, nck = u // 2, u % 2
            tt = qc * 4 + tti
            tsl = slice(tt * 128, (tt + 1) * 128)
            nsl = slice(nck * 512, (nck + 1) * 512)
            po = ps.tile(
                [128, 512], F32, name="po", tag=tag, bufs=(4 if tag == "po" else 2)
            )
            for dt2 in range(N_DT):
                nc.tensor.matmul(
                    po[:], cxt[dt2][:, tsl], wo_k(dt2)[:, nsl],
                    start=(dt2 == 0), stop=(dt2 == 3),
                )
            ob = outp.tile([128, 512], F32, name="ob", tag="ob")
            if copy_engine == "scalar":
                nc.scalar.copy(ob[:], po[:])
            else:
                nc.vector.tensor_copy(ob[:], po[:])
            nc.sync.dma_start(out_d.ap()[tsl, nsl], ob[:])

        # out-proj unit of the final chunk, split: dt2=0..2 accumulate inside
        # the last attention block (psum tile stays open), dt2=3 + copyback
        # after that block's normalization.
        def start_unit3(u):
            tti, nck = u // 2, u % 2
            tt = 12 + tti
            tsl = slice(tt * 128, (tt + 1) * 128)
            nsl = slice(nck * 512, (nck + 1) * 512)
            po = ps.tile([128, 512], F32, name="po3", tag="po", bufs=4)
            for dt2 in range(3):
                nc.tensor.matmul(
                    po[:], cxt[dt2][:, tsl], wo_k(dt2)[:, nsl],
                    start=(dt2 == 0), stop=False,
                )
            return (po, tsl, nsl)

        def finish_unit3(st, copy_engine="vector"):
            po, tsl, nsl = st
            nc.tensor.matmul(po[:], cxt[3][:, tsl], wo_k(3)[:, nsl], start=False, stop=True)
            ob = outp.tile([128, 512], F32, name="ob", tag="ob")
            if copy_engine == "scalar":
                nc.scalar.copy(ob[:], po[:])
            else:
                nc.vector.tensor_copy(ob[:], po[:])
            nc.sync.dma_start(out_d.ap()[tsl, nsl], ob[:])

        # ---- emission schedule (v1) ----
        emit_proj(0)
        for qc in range(N_QC):
            for dt in range(N_DT):
                emit_attn_dt(qc, dt)
                if qc > 0:
                    emit_outproj_unit(qc - 1, 2 * dt)
                    emit_outproj_unit(qc - 1, 2 * dt + 1)
                if qc + 1 < N_QC:
                    emit_proj_piece(qc + 1, dt)
        for u in range(8):
            emit_outproj_unit(3, u, tag=("ps" if u % 2 else "po"))


def build_bass():
    nc = bacc.Bacc("TRN2", target_bir_lowering=False, debug=False, num_devices=N_CORES)
    xt_d = nc.dram_tensor("xt", (D, N), BF16, kind="ExternalInput")
    wq_d = nc.dram_tensor("wq", (D, DH), BF16, kind="ExternalInput")
    wk_d = nc.dram_tensor("wk", (D, DH), BF16, kind="ExternalInput")
    wv_d = nc.dram_tensor("wv", (D, DH), BF16, kind="ExternalInput")
    wo_d = nc.dram_tensor("wo", (DH, D), BF16, kind="ExternalInput")
    out_d = nc.dram_tensor("out", (N, D), F32, kind="ExternalOutput")
    with tile.TileContext(nc) as tc:
        _emit(nc, tc, xt_d, wq_d, wk_d, wv_d, wo_d, out_d)
    nc.compile()
    return nc


_NC = None


def _get_nc():
    global _NC
    if _NC is None:
        _NC = build_bass()
    return _NC


def make_in_maps(x, Wq, Wk, Wv, Wo):
    bf = ml_dtypes.bfloat16
    in_maps = []
    for c in range(N_CORES):
        b, g = c // 2, c % 2
        gs = slice(g * DH, (g + 1) * DH)
        in_maps.append(
            {
                "xt": np.ascontiguousarray(x[b].T).astype(bf),
                "wq": np.ascontiguousarray(Wq[:, gs]).astype(bf),
                "wk": np.ascontiguousarray(Wk[:, gs]).astype(bf),
                "wv": np.ascontiguousarray(Wv[:, gs]).astype(bf),
                "wo": np.ascontiguousarray(Wo[gs, :]).astype(bf),
            }
        )
    return in_maps


def kernel(x, Wq, Wk, Wv, Wo, bo, _trace=False):
    x = np.asarray(x, dtype=np.float32)
    nc = _get_nc()
    in_maps = make_in_maps(x, Wq, Wk, Wv, Wo)
    res = bass_utils.run_bass_kernel_spmd(
        nc, in_maps, core_ids=list(range(N_CORES)), trace=_trace
    )
    out = np.empty((B, N, D), dtype=np.float32)
    bo32 = np.asarray(bo, dtype=np.float32)
    for b in range(B):
        out[b] = res.results[2 * b]["out"] + res.results[2 * b + 1]["out"] + bo32
    if _trace:
        return out, res
    return out
